# revision 1
# baseline (speedup 1.0000x reference)
"""BiRNN (bidirectional LSTM) encoder kernel for Trainium2, 8-core SPMD.

Problem: input_w [32, 32, 64] int token ids -> emb lookup [1024, 64, 512]
-> forward + backward LSTM (hidden 512 each) -> concat [1024, 64, 1024]
-> max over time -> [32, 32, 1024].

Sharding: data-parallel over the 1024 = 32*32 sequences, 128 per core.
Weights + embedding table replicated. No collectives needed.

Per-core kernel: for each time step and each direction,
  - indirect-DMA gather x_t = emb[idx[:, t]]            [128 seq, 512]
  - PE-transpose x_t -> xT (E on partitions)
  - gates psum[seq, 2048] = xT.T @ W_ih^T + hT.T @ W_hh^T   (float32r MMs)
  - += bias (DVE, in-psum), sigmoid/tanh (ACT, reads psum)
  - LSTM cell elementwise (DVE), running max of h
  - PE-transpose h -> hT for next step
"""

import sys

for _p in ("/opt/trn_rl_repo",):
    if _p not in sys.path:
        sys.path.append(_p)

import numpy as np

import concourse.bass as bass
import concourse.bacc as bacc
import concourse.mybir as mybir
import concourse.tile as tile
from concourse.bass_utils import run_bass_kernel_spmd
from concourse.masks import make_identity

V, E, HID = 32000, 512, 1024
HD = HID // 2          # per-direction hidden = 512
G = 4 * HD             # gates per direction = 2048
T = 64                 # sequence length
NCORES = 8
NSEQ = 32 * 32         # total sequences
S = NSEQ // NCORES     # 128 sequences per core
KC = E // 128          # 4 contraction chunks (E == HD == 512)

F32 = mybir.dt.float32
F32R = mybir.dt.float32r
I32 = mybir.dt.int32
AF = mybir.ActivationFunctionType
ALU = mybir.AluOpType

LAST_RESULTS = None


def _emit(tc, out_d, idx_d, emb_d, wihT_d, whhT_d, bias_d):
    nc = tc.nc

    with (
        tc.tile_pool(name="const", bufs=1) as cpool,
        tc.tile_pool(name="state", bufs=1) as spool,
        tc.tile_pool(name="xio", bufs=3) as xpool,
        tc.tile_pool(name="work", bufs=2) as wpool,
        tc.tile_pool(name="pgates", bufs=2, space="PSUM") as pg,
        tc.tile_pool(name="ptr", bufs=2, space="PSUM") as pt,
    ):
        # ---- constants ----
        # token ids + bias first: tiny, and the t=0 gathers depend on idx
        idx_sb = cpool.tile([128, T], I32)
        nc.sync.dma_start(idx_sb[:, :], idx_d[:, :])
        bias_sb = cpool.tile([128, 2 * G], F32)
        nc.sync.dma_start(bias_sb[:, :], bias_d[:, :])
        # W_ih^T for both dirs: [E, 2G] -> [128, KC, 2G]
        wih_sb = cpool.tile([128, KC, 2 * G], F32R)
        wihT_r = wihT_d[:, :].rearrange("(c p) g -> p c g", p=128)
        for dh in range(2):
            for k in range(KC):
                nc.sync.dma_start(wih_sb[:, k, dh * G:(dh + 1) * G], wihT_r[:, k, dh * G:(dh + 1) * G])
        # W_hh^T per dir: [2, HD, G] -> [128, 2, KC, G]
        whh_sb = cpool.tile([128, 2, KC, G], F32R)
        whhT_r = whhT_d[:, :, :].rearrange("d (c p) g -> p d c g", p=128)
        for d in range(2):
            for k in range(KC):
                nc.sync.dma_start(whh_sb[:, d, k, :], whhT_r[:, d, k, :])
        ident_f = cpool.tile([128, 128], F32)
        make_identity(nc, ident_f[:, :])
        ident = cpool.tile([128, 128], F32R)
        nc.vector.tensor_copy(ident[:, :], ident_f[:, :])

        # ---- state ----
        # hT: h transposed, [HD on partitions] per dir: [128, 2, KC, 128]
        hT_sb = spool.tile([128, 2, KC, 128], F32R)
        c_sb = spool.tile([128, 2, HD], F32)
        hmax_sb = spool.tile([128, 2, HD], F32)
        nc.vector.memset(c_sb[:, :, :], 0.0)

        h_prev_tiles = {}
        for t in range(T):
            for d in ((1, 0) if t == T - 1 else (0, 1)):
                td = t if d == 0 else (T - 1 - t)

                # transpose h(t-1) for this dir (emitted here so PE never
                # head-of-line blocks: the other dir's MMs ran in between)
                if t > 0:
                    hT_ps = pt.tile([128, HD], F32R)
                    h_prev = h_prev_tiles[d]
                    for k in range(KC):
                        nc.tensor.transpose(
                            hT_ps[:, k * 128:(k + 1) * 128],
                            h_prev[:, k * 128:(k + 1) * 128],
                            ident[:, :],
                        )
                    nc.vector.tensor_copy(hT_sb[:, d, :, :], hT_ps[:, :].rearrange("p (c q) -> p c q", c=KC))

                # gather x_t and transpose
                x32 = xpool.tile([128, E], F32R)
                nc.gpsimd.indirect_dma_start(
                    out=x32[:, :],
                    out_offset=None,
                    in_=emb_d[:, :],
                    in_offset=bass.IndirectOffsetOnAxis(ap=idx_sb[:, td:td + 1], axis=0),
                )
                xT_ps = pt.tile([128, E], F32R)
                for k in range(KC):
                    nc.tensor.transpose(
                        xT_ps[:, k * 128:(k + 1) * 128],
                        x32[:, k * 128:(k + 1) * 128],
                        ident[:, :],
                    )
                xT_sb = xpool.tile([128, E], F32R)
                nc.vector.tensor_copy(xT_sb[:, :], xT_ps[:, :])

                # gates = x W_ih^T + h W_hh^T + b, psum [128 seq, 2048]
                # processed as 2 halves of [128, 1024] (2 psum banks each)
                acts = wpool.tile([128, G], F32)
                gbase = d * G
                for hh in range(2):
                    gp = pg.tile([128, 1024], F32)
                    for b in range(2):
                        n0 = gbase + hh * 1024 + b * 512
                        for k in range(KC):
                            nc.tensor.matmul(
                                gp[:, b * 512:(b + 1) * 512],
                                xT_sb[:, k * 128:(k + 1) * 128],
                                wih_sb[:, k, n0:n0 + 512],
                                start=(k == 0),
                                stop=(t == 0 and k == KC - 1),
                            )
                        if t > 0:
                            hn0 = hh * 1024 + b * 512
                            for k in range(KC):
                                nc.tensor.matmul(
                                    gp[:, b * 512:(b + 1) * 512],
                                    hT_sb[:, d, k, :],
                                    whh_sb[:, d, k, hn0:hn0 + 512],
                                    start=False,
                                    stop=(k == KC - 1),
                                )
                    # bias add in-psum, then activations read psum
                    nc.vector.tensor_add(
                        gp[:, :],
                        gp[:, :],
                        bias_sb[:, gbase + hh * 1024: gbase + (hh + 1) * 1024],
                    )
                    if hh == 0:
                        # cols 0:1024 = i|f -> sigmoid
                        nc.scalar.activation(acts[:, 0:1024], gp[:, :], AF.Sigmoid)
                    else:
                        # cols 1024:1536 = g -> tanh; 1536:2048 = o -> sigmoid
                        nc.scalar.activation(acts[:, 1024:1536], gp[:, 0:512], AF.Tanh)
                        nc.scalar.activation(acts[:, 1536:2048], gp[:, 512:1024], AF.Sigmoid)

                # cell update (DVE + ACT)
                h_sb = wpool.tile([128, HD], F32R)
                t1 = wpool.tile([128, HD], F32)
                tanh_c = wpool.tile([128, HD], F32)
                # t1 = i * tanh(g)
                nc.vector.tensor_mul(t1[:, :], acts[:, 0:512], acts[:, 1024:1536])
                # c = f * c + t1
                nc.vector.tensor_mul(c_sb[:, d, :], acts[:, 512:1024], c_sb[:, d, :])
                nc.vector.tensor_add(c_sb[:, d, :], c_sb[:, d, :], t1[:, :])
                nc.scalar.activation(tanh_c[:, :], c_sb[:, d, :], AF.Tanh)
                # h = o * tanh(c)
                nc.vector.tensor_mul(h_sb[:, :], acts[:, 1536:2048], tanh_c[:, :])
                # running max over time
                if t == 0:
                    nc.vector.tensor_copy(hmax_sb[:, d, :], h_sb[:, :])
                else:
                    nc.vector.tensor_max(hmax_sb[:, d, :], hmax_sb[:, d, :], h_sb[:, :])

                # stash handle for the hT transpose emitted next iteration
                h_prev_tiles[d] = h_sb

        # write out [128, 1024] = [hmax_f | hmax_b]
        nc.sync.dma_start(out_d[:, 0:HD], hmax_sb[:, 0, :])
        nc.sync.dma_start(out_d[:, HD:HID], hmax_sb[:, 1, :])


_CACHED = None


def _build():
    global _CACHED
    if _CACHED is not None:
        return _CACHED
    nc = bacc.Bacc("TRN2", target_bir_lowering=False)
    idx_d = nc.dram_tensor("idx", [S, T], I32, kind="ExternalInput")
    emb_d = nc.dram_tensor("emb", [V, E], F32R, kind="ExternalInput")
    wihT_d = nc.dram_tensor("wihT", [E, 2 * G], F32R, kind="ExternalInput")
    whhT_d = nc.dram_tensor("whhT", [2, HD, G], F32R, kind="ExternalInput")
    bias_d = nc.dram_tensor("bias", [128, 2 * G], F32, kind="ExternalInput")
    out_d = nc.dram_tensor("out", [S, HID], F32, kind="ExternalOutput")
    with tile.TileContext(nc) as tc:
        _emit(tc, out_d, idx_d, emb_d, wihT_d, whhT_d, bias_d)
    nc.compile()
    _CACHED = nc
    return nc


def _run(inputs, trace=False, **run_kwargs):
    global LAST_RESULTS
    idx = np.ascontiguousarray(np.asarray(inputs["input_w"]).reshape(NSEQ, T).astype(np.int32))
    emb = np.ascontiguousarray(np.asarray(inputs["emb"], dtype=np.float32))
    wihT = np.ascontiguousarray(
        np.concatenate(
            [np.asarray(inputs["w_ih_f"], dtype=np.float32), np.asarray(inputs["w_ih_b"], dtype=np.float32)],
            axis=0,
        ).T
    )  # [E, 2G]
    whhT = np.ascontiguousarray(
        np.stack(
            [np.asarray(inputs["w_hh_f"], dtype=np.float32).T, np.asarray(inputs["w_hh_b"], dtype=np.float32).T],
            axis=0,
        )
    )  # [2, HD, G]
    bias = np.concatenate(
        [np.asarray(inputs["b_f"], dtype=np.float32), np.asarray(inputs["b_b"], dtype=np.float32)]
    )
    bias_tiled = np.ascontiguousarray(np.tile(bias[None, :], (128, 1)))

    nc = _build()
    in_maps = []
    for i in range(NCORES):
        in_maps.append(
            {
                "idx": idx[i * S:(i + 1) * S],
                "emb": emb,
                "wihT": wihT,
                "whhT": whhT,
                "bias": bias_tiled,
            }
        )
    res = run_bass_kernel_spmd(nc, in_maps, core_ids=list(range(NCORES)), trace=trace, **run_kwargs)
    LAST_RESULTS = res
    out = np.concatenate([res.results[i]["out"] for i in range(NCORES)], axis=0)
    return out.reshape(32, 32, HID).astype(np.float32)


def kernel(**inputs):
    return _run(inputs, trace=False)


# ---------------------------------------------------------------------------
# Timing-only path (test harness): reusable jitted executable, inputs
# device-resident, no donation, so repeated calls measure NEFF exec time.
# ---------------------------------------------------------------------------

def _prep_in_maps(inputs):
    idx = np.ascontiguousarray(np.asarray(inputs["input_w"]).reshape(NSEQ, T).astype(np.int32))
    emb = np.ascontiguousarray(np.asarray(inputs["emb"], dtype=np.float32))
    wihT = np.ascontiguousarray(
        np.concatenate(
            [np.asarray(inputs["w_ih_f"], dtype=np.float32), np.asarray(inputs["w_ih_b"], dtype=np.float32)],
            axis=0,
        ).T
    )
    whhT = np.ascontiguousarray(
        np.stack(
            [np.asarray(inputs["w_hh_f"], dtype=np.float32).T, np.asarray(inputs["w_hh_b"], dtype=np.float32).T],
            axis=0,
        )
    )
    bias = np.concatenate(
        [np.asarray(inputs["b_f"], dtype=np.float32), np.asarray(inputs["b_b"], dtype=np.float32)]
    )
    bias_tiled = np.ascontiguousarray(np.tile(bias[None, :], (128, 1)))
    return [
        {
            "idx": idx[i * S:(i + 1) * S],
            "emb": emb,
            "wihT": wihT,
            "whhT": whhT,
            "bias": bias_tiled,
        }
        for i in range(NCORES)
    ]


def timed_run(inputs, iters=5):
    """Returns (output, per_call_seconds_list). Inputs put on device once."""
    import time

    import jax
    from jax.sharding import Mesh, PartitionSpec
    from jax.experimental.shard_map import shard_map

    from concourse import bass2jax

    nc = _build()
    bass2jax.install_neuronx_cc_hook()
    partition_name = nc.partition_id_tensor.name if nc.partition_id_tensor else None
    in_names, out_names, out_avals = [], [], []
    for alloc in nc.m.functions[0].allocations:
        if not isinstance(alloc, mybir.MemoryLocationSet):
            continue
        name = alloc.memorylocations[0].name
        if alloc.kind == "ExternalInput":
            if name != partition_name:
                in_names.append(name)
        elif alloc.kind == "ExternalOutput":
            out_avals.append(
                jax.core.ShapedArray(tuple(alloc.tensor_shape), mybir.dt.np(alloc.dtype))
            )
            out_names.append(name)

    n_params = len(in_names)
    all_in_names = list(in_names) + list(out_names)
    if partition_name is not None:
        all_in_names.append(partition_name)

    def _body(*args):
        operands = list(args)
        if partition_name is not None:
            operands.append(bass2jax.partition_id_tensor())
        outs = bass2jax._bass_exec_p.bind(
            *operands,
            out_avals=tuple(out_avals),
            in_names=tuple(all_in_names),
            out_names=tuple(out_names),
            lowering_input_output_aliases=(),
            sim_require_finite=True,
            sim_require_nnan=True,
            nc=nc,
        )
        return tuple(outs)

    devices = jax.devices()[:NCORES]
    mesh = Mesh(np.asarray(devices), ("core",))
    n_outs = len(out_names)
    in_specs = (PartitionSpec("core"),) * (n_params + n_outs)
    out_specs = (PartitionSpec("core"),) * n_outs
    sharded = jax.jit(
        shard_map(_body, mesh=mesh, in_specs=in_specs, out_specs=out_specs, check_rep=False)
    )

    in_maps = _prep_in_maps(inputs)
    concat_in = [
        np.concatenate([np.asarray(in_maps[c][nm]) for c in range(NCORES)], axis=0)
        for nm in in_names
    ]
    concat_zeros = [
        np.zeros((NCORES * a.shape[0], *a.shape[1:]), a.dtype) for a in out_avals
    ]
    from jax.sharding import NamedSharding

    shard = NamedSharding(mesh, PartitionSpec("core"))
    dev_args = [jax.device_put(a, shard) for a in concat_in + concat_zeros]
    out = sharded(*dev_args)
    jax.block_until_ready(out)

    times = []
    for _ in range(iters):
        t0 = time.perf_counter()
        out = sharded(*dev_args)
        jax.block_until_ready(out)
        times.append(time.perf_counter() - t0)

    full = np.concatenate(
        [np.asarray(out[out_names.index("out")]).reshape(NCORES, S, HID)[c] for c in range(NCORES)],
        axis=0,
    )
    return full.reshape(32, 32, HID).astype(np.float32), times



# revision 38
# speedup vs baseline: 1.3530x; 1.3530x over previous
"""BiRNN (bidirectional LSTM) encoder kernel for Trainium2, 8-core SPMD.

Problem: input_w [32, 32, 64] int token ids -> emb lookup [1024, 64, 512]
-> forward + backward LSTM (hidden 512 each) -> concat [1024, 64, 1024]
-> max over time -> [32, 32, 1024].

Sharding: data-parallel over the 1024 = 32*32 sequences, 128 per core.
Weights + embedding table replicated. No collectives needed.

Mixed precision: gate columns reordered to [i|f|o|g] per direction.
i/f/o pre-activations computed in fp8-e4m3 with DoubleRow matmuls
(2x PE MAC rate); the error-critical g gate in bf16 (1 cyc/row).
Scales: x64 on both fp8 operands, psum scale 4096 (descale fused
into the activations). Cell state c in fp32, h stored bf16.
Numerically validated: rel ~8e-3 vs fp32 reference (gate: 2e-2).

Per core: x gathered as bf16 rows of emb, PE-transposed once per t
(shared across directions, software-pipelined 2 steps ahead), cast into
bf16 + fp8 SBUF caches. Per step/dir: 12 fp8-DR + 8 bf16 matmuls
into psum tiles [i|f], [o], [g]; activations with fused descale;
cell elementwise spread over DVE/Pool; h transposed on PE in bf16.
PSUM: gates use all 8 banks (2 dirs in flight); the per-step h/x
transposes borrow the g-tile's bytes (bitcast bf16) before the g
matmuls reset the bank.
"""

import sys

for _p in ("/opt/trn_rl_repo",):
    if _p not in sys.path:
        sys.path.append(_p)

import numpy as np
import ml_dtypes

import concourse.bass as bass
import concourse.bacc as bacc
import concourse.mybir as mybir
import concourse.tile as tile
from concourse.bass_utils import run_bass_kernel_spmd
from concourse.masks import make_identity

V, E, HID = 32000, 512, 1024
HD = HID // 2          # per-direction hidden = 512
T = 64                 # sequence length
NCORES = 8
NSEQ = 32 * 32
S = NSEQ // NCORES     # 128 sequences per core
KC = E // 128          # 4 contraction chunks
NIFO = 3 * HD          # 1536 i|f|o cols per dir

F32 = mybir.dt.float32
F32R = mybir.dt.float32r
BF16 = mybir.dt.bfloat16
FP8 = mybir.dt.float8e4
I32 = mybir.dt.int32
AF = mybir.ActivationFunctionType
PM = mybir.MatmulPerfMode

QSCALE = 64.0          # fp8 operand scale; psum scale = 64*64
DESCALE = 1.0 / (QSCALE * QSCALE)

LAST_RESULTS = None


def _emit(tc, out_d, idx_d, emb16_d, w8_d, wg16_d, wh8_d, whg16_d, bias_d, use_bias):
    nc = tc.nc

    with (
        tc.tile_pool(name="const", bufs=1) as cpool,
        tc.tile_pool(name="xcache", bufs=1) as xc,
        tc.tile_pool(name="state", bufs=1) as spool,
        tc.tile_pool(name="xio", bufs=6) as xpool,
        tc.tile_pool(name="acts", bufs=3) as apool,
        tc.tile_pool(name="cell", bufs=4) as wpool,
        tc.tile_pool(name="hbuf", bufs=3) as hpool,
    ):
        # ---- constants ----
        idx_sb = cpool.tile([128, T], I32)
        nc.sync.dma_start(idx_sb[:, :], idx_d[:, :])
        if use_bias:
            bias_sb = cpool.tile([128, 2, 2048], F32)
            nc.sync.dma_start(bias_sb[:, :, :], bias_d[:, :, :])
        w8_sb = cpool.tile([128, 2, 2, 2 * NIFO], FP8)
        nc.sync.dma_start(w8_sb[:, :, :, :], w8_d[:, :, :, :])
        wh8_sb = cpool.tile([128, 2, 2, 2 * NIFO], FP8)
        nc.sync.dma_start(wh8_sb[:, :, :, :], wh8_d[:, :, :, :])
        wg16_sb = cpool.tile([128, KC, 2 * HD], BF16)
        nc.sync.dma_start(wg16_sb[:, :, :], wg16_d[:, :, :])
        whg16_sb = cpool.tile([128, KC, 2 * HD], BF16)
        nc.sync.dma_start(whg16_sb[:, :, :], whg16_d[:, :, :])
        ident_f = cpool.tile([128, 128], F32)
        make_identity(nc, ident_f[:, :])
        ident16 = cpool.tile([128, 128], BF16)
        nc.vector.tensor_copy(ident16[:, :], ident_f[:, :])

        # ---- x caches (shared across directions) ----
        xT16 = xc.tile([128, T, KC, 128], BF16)
        xT8 = xc.tile([128, T, KC, 128], FP8)

        # ---- state ----
        c_sb = spool.tile([128, 2, HD], F32)
        nc.vector.memset(c_sb[:, :, :], 0.0)
        hmax_sb = spool.tile([128, 2, HD], BF16)
        hT16 = spool.tile([128, 2, KC, 128], BF16)
        hT8 = spool.tile([128, 2, KC, 128], FP8)

        pending_x16 = {}

        def emit_gather(t):
            """issue the token-column gather early (gpsimd queue order)"""
            x16 = xpool.tile([128, E], BF16)
            nc.gpsimd.indirect_dma_start(
                out=x16[:, :],
                out_offset=None,
                in_=emb16_d[:, :],
                in_offset=bass.IndirectOffsetOnAxis(ap=idx_sb[:, t:t + 1], axis=0),
            )
            pending_x16[t] = x16

        def emit_tp(t, xps):
            """transpose gathered column t into xps psum view, cast to caches"""
            x16 = pending_x16.pop(t)
            for k in range(KC):
                nc.tensor.transpose(
                    xps[:, k * 128:(k + 1) * 128],
                    x16[:, k * 128:(k + 1) * 128],
                    ident16[:, :],
                )
            xps_v = xps.rearrange("p (c q) -> p c q", c=KC)
            nc.scalar.activation(xT16[:, t, :, :], xps_v, AF.Copy)
            nc.vector.tensor_scalar_mul(xT8[:, t, :, :], xps_v, QSCALE)

        # prologue: x for the first two steps of both directions, plus
        # gathers for the pair transposed at sd(0, 0)
        with tc.tile_pool(name="pro", bufs=2, space="PSUM") as pro:
            for t in (0, 63, 1, 62):
                emit_gather(t)
                xps = pro.tile([128, E], BF16)
                emit_tp(t, xps[:, :])
            emit_gather(2)
            emit_gather(61)

        with (
            tc.tile_pool(name="pgA", bufs=2, space="PSUM") as pgA,
            tc.tile_pool(name="pgO", bufs=2, space="PSUM") as pgO,
            tc.tile_pool(name="pgG", bufs=2, space="PSUM") as pgG,
        ):
            h_prev = {}

            def emit_cell(t, d, acts):
                """cell update for sd (t, d); tail split into hd-halves so
                the first half's h is ready for its transpose sooner"""
                t1 = wpool.tile([128, HD], F32)
                nc.vector.tensor_mul(t1[:, :], acts[:, 0:512], acts[:, 1536:2048])
                nc.gpsimd.tensor_mul(c_sb[:, d, :], acts[:, 512:1024], c_sb[:, d, :])
                nc.gpsimd.tensor_add(c_sb[:, d, :], c_sb[:, d, :], t1[:, :])
                tnh = wpool.tile([128, HD], F32)
                h16 = hpool.tile([128, HD], BF16)
                for hh in range(2):
                    sl = slice(hh * 256, (hh + 1) * 256)
                    nc.scalar.activation(tnh[:, sl], c_sb[:, d, sl], AF.Tanh)
                    nc.vector.tensor_mul(h16[:, sl], acts[:, 1024 + hh * 256:1280 + hh * 256], tnh[:, sl])
                if t == 0:
                    nc.vector.tensor_copy(hmax_sb[:, d, :], h16[:, :])
                else:
                    nc.vector.tensor_max(hmax_sb[:, d, :], hmax_sb[:, d, :], h16[:, :])
                h_prev[d] = h16

            for t in range(T):
                for d in (0, 1):
                    td = t if d == 0 else (T - 1 - t)
                    c0 = d * NIFO          # ifo col base for this dir
                    g0 = d * HD            # g col base

                    # gather for the entry transposed next sd of this dir
                    # (one block-pair of DMA lead time)
                    if t <= 28:
                        emit_gather(t + 3 if d == 0 else 60 - t)

                    hA = pgA.tile([128, 1024], F32)   # i|f
                    hO = pgO.tile([128, 512], F32)    # o
                    hG = pgG.tile([128, 512], F32)    # g
                    scr = hG[:, :].bitcast(BF16)      # [128,1024] scratch for tp
                    scrO = hO[:, :].bitcast(BF16)

                    # x pipeline at block top: the gather is a block old so
                    # nothing stalls, and its ACT copy precedes the sigmoids
                    if t <= 29:
                        if d == 0:
                            emit_tp(t + 2, scr[:, 512:1024])
                        else:
                            emit_tp(61 - t, scrO[:, 0:512])

                    # x-part matmuls for i|f (no h dependency)
                    for nb in range(2):
                        for j in range(2):
                            nc.tensor.matmul(
                                hA[:, nb * 512:(nb + 1) * 512],
                                xT8[:, td, 2 * j:2 * j + 2, :],
                                w8_sb[:, j, :, c0 + nb * 512:c0 + (nb + 1) * 512],
                                start=(j == 0), stop=(t == 0 and j == 1),
                                perf_mode=PM.DoubleRow,
                            )
                    for j in range(2):
                        nc.tensor.matmul(
                            hO[:, :],
                            xT8[:, td, 2 * j:2 * j + 2, :],
                            w8_sb[:, j, :, c0 + 1024:c0 + 1536],
                            start=(j == 0), stop=(t == 0 and j == 1),
                            perf_mode=PM.DoubleRow,
                        )

                    # transpose h(t-1, d) into g-tile scratch, then cast
                    if t > 0:
                        hp = h_prev[d]
                        for k in range(KC):
                            nc.tensor.transpose(
                                scr[:, k * 128:(k + 1) * 128],
                                hp[:, k * 128:(k + 1) * 128],
                                ident16[:, :],
                            )
                        hps_v = scr[:, 0:512].rearrange("p (c q) -> p c q", c=KC)
                        # halves so the first h-matmuls start one cast earlier;
                        # all on DVE so the ACT queue never blocks on the tp
                        nc.vector.tensor_scalar_mul(hT8[:, d, 0:2, :], hps_v[:, 0:2, :], QSCALE)
                        nc.vector.tensor_copy(hT16[:, d, 0:2, :], hps_v[:, 0:2, :])
                        nc.vector.tensor_scalar_mul(hT8[:, d, 2:4, :], hps_v[:, 2:4, :], QSCALE)
                        nc.vector.tensor_copy(hT16[:, d, 2:4, :], hps_v[:, 2:4, :])

                    # g-gate x-part (start resets the borrowed bank)
                    for k in range(KC):
                        nc.tensor.matmul(
                            hG[:, :],
                            xT16[:, td, k, :],
                            wg16_sb[:, k, g0:g0 + 512],
                            start=(k == 0), stop=(t == 0 and k == KC - 1),
                        )

                    # h-part matmuls
                    if t > 0:
                        for nb in range(2):
                            for j in range(2):
                                nc.tensor.matmul(
                                    hA[:, nb * 512:(nb + 1) * 512],
                                    hT8[:, d, 2 * j:2 * j + 2, :],
                                    wh8_sb[:, j, :, c0 + nb * 512:c0 + (nb + 1) * 512],
                                    start=False, stop=(j == 1),
                                    perf_mode=PM.DoubleRow,
                                )
                        for j in range(2):
                            nc.tensor.matmul(
                                hO[:, :],
                                hT8[:, d, 2 * j:2 * j + 2, :],
                                wh8_sb[:, j, :, c0 + 1024:c0 + 1536],
                                start=False, stop=(j == 1),
                                perf_mode=PM.DoubleRow,
                            )
                        for k in range(KC):
                            nc.tensor.matmul(
                                hG[:, :],
                                hT16[:, d, k, :],
                                whg16_sb[:, k, g0:g0 + 512],
                                start=False, stop=(k == KC - 1),
                            )

                    if use_bias:
                        nc.vector.tensor_add(hA[:, :], hA[:, :], bias_sb[:, d, 0:1024])
                        nc.vector.tensor_add(hO[:, :], hO[:, :], bias_sb[:, d, 1024:1536])
                        nc.vector.tensor_add(hG[:, :], hG[:, :], bias_sb[:, d, 1536:2048])

                    # activations; acts layout [i|f|o|g] fp32.
                    # tanh(g) before sigmoid(o): the cell chain needs g early,
                    # o only at the final h multiply.
                    acts = apool.tile([128, 2048], F32)
                    nc.scalar.activation(acts[:, 0:1024], hA[:, :], AF.Sigmoid, scale=DESCALE)
                    nc.scalar.activation(acts[:, 1536:2048], hG[:, :], AF.Tanh)
                    nc.scalar.activation(acts[:, 1024:1536], hO[:, :], AF.Sigmoid, scale=DESCALE)

                    emit_cell(t, d, acts)

        # write out [128, 1024] = [hmax_f | hmax_b], cast bf16 -> f32
        outf = apool.tile([128, HID], F32)
        nc.vector.tensor_copy(outf[:, :].rearrange("p (d q) -> p d q", d=2), hmax_sb[:, :, :])
        nc.sync.dma_start(out_d[:, :], outf[:, :])


_CACHED = {}


def _build(use_bias):
    if use_bias in _CACHED:
        return _CACHED[use_bias]
    nc = bacc.Bacc("TRN2", target_bir_lowering=False)
    idx_d = nc.dram_tensor("idx", [S, T], I32, kind="ExternalInput")
    emb16_d = nc.dram_tensor("emb16", [V, E], BF16, kind="ExternalInput")
    w8_d = nc.dram_tensor("w8", [128, 2, 2, 2 * NIFO], FP8, kind="ExternalInput")
    wh8_d = nc.dram_tensor("wh8", [128, 2, 2, 2 * NIFO], FP8, kind="ExternalInput")
    wg16_d = nc.dram_tensor("wg16", [128, KC, 2 * HD], BF16, kind="ExternalInput")
    whg16_d = nc.dram_tensor("whg16", [128, KC, 2 * HD], BF16, kind="ExternalInput")
    bias_d = None
    if use_bias:
        bias_d = nc.dram_tensor("bias", [128, 2, 2048], F32, kind="ExternalInput")
    out_d = nc.dram_tensor("out", [S, HID], F32, kind="ExternalOutput")
    with tile.TileContext(nc) as tc:
        _emit(tc, out_d, idx_d, emb16_d, w8_d, wg16_d, wh8_d, whg16_d, bias_d, use_bias)
    nc.compile()
    _CACHED[use_bias] = nc
    return nc


def _prep(inputs):
    """Host-side packing of weights into kernel layouts."""
    idx = np.ascontiguousarray(
        np.asarray(inputs["input_w"]).reshape(NSEQ, T).astype(np.int32))
    emb16 = np.ascontiguousarray(
        np.asarray(inputs["emb"], dtype=np.float32).astype(ml_dtypes.bfloat16))

    def pack(wih, whh):
        # w: [4*HD, K] rows in i,f,g,o order -> ifo fp8 [128,2,2,1536], g bf16 [128,KC,512]
        out8, outg = [], []
        for w, ws in ((wih, QSCALE), (whh, QSCALE)):
            w = np.asarray(w, dtype=np.float32)
            ifo = np.concatenate([w[0:1024], w[1536:2048]], axis=0)  # i,f,o [1536,K]
            g = w[1024:1536]                                         # [512,K]
            t8 = (ifo.T * ws).reshape(2, 2, 128, NIFO).astype(ml_dtypes.float8_e4m3)
            out8.append(np.transpose(t8, (2, 0, 1, 3)))              # [128,2,2,1536]
            tg = g.T.reshape(KC, 128, HD).astype(ml_dtypes.bfloat16)
            outg.append(np.transpose(tg, (1, 0, 2)))                 # [128,KC,512]
        return out8, outg

    (w8f, wh8f), (wgf, whgf) = pack(inputs["w_ih_f"], inputs["w_hh_f"])
    (w8b, wh8b), (wgb, whgb) = pack(inputs["w_ih_b"], inputs["w_hh_b"])
    w8 = np.ascontiguousarray(np.concatenate([w8f, w8b], axis=3))
    wh8 = np.ascontiguousarray(np.concatenate([wh8f, wh8b], axis=3))
    wg16 = np.ascontiguousarray(np.concatenate([wgf, wgb], axis=2))
    whg16 = np.ascontiguousarray(np.concatenate([whgf, whgb], axis=2))

    bias = np.zeros((2, 2048), np.float32)
    use_bias = False
    for d, bname in enumerate(["b_f", "b_b"]):
        b = np.asarray(inputs[bname], dtype=np.float32)
        if np.any(b != 0):
            use_bias = True
        # reorder i,f,g,o -> i,f,o,g ; prescale i,f,o by the psum scale
        bias[d, 0:1024] = b[0:1024] * (QSCALE * QSCALE)
        bias[d, 1024:1536] = b[1536:2048] * (QSCALE * QSCALE)
        bias[d, 1536:2048] = b[1024:1536]
    bias_tiled = np.ascontiguousarray(np.broadcast_to(bias[None], (128, 2, 2048)).copy())

    in_maps = []
    for i in range(NCORES):
        m = {
            "idx": idx[i * S:(i + 1) * S],
            "emb16": emb16,
            "w8": w8, "wh8": wh8, "wg16": wg16, "whg16": whg16,
        }
        if use_bias:
            m["bias"] = bias_tiled
        in_maps.append(m)
    return in_maps, use_bias


def _run(inputs, trace=False, **run_kwargs):
    global LAST_RESULTS
    in_maps, use_bias = _prep(inputs)
    nc = _build(use_bias)
    res = run_bass_kernel_spmd(nc, in_maps, core_ids=list(range(NCORES)),
                               trace=trace, **run_kwargs)
    LAST_RESULTS = res
    out = np.concatenate([res.results[i]["out"] for i in range(NCORES)], axis=0)
    return out.reshape(32, 32, HID).astype(np.float32)


def kernel(**inputs):
    return _run(inputs, trace=False)
